# revision 78
# baseline (speedup 1.0000x reference)
"""Single-head causal attention (B=4, S=4096, E=512, DK=DV=64) on 8 trn2 cores.

Sharding: 2 cores per batch element; each core owns 4 q-groups of 512 rows.
Core role X (natural key order) owns rows 1024g..1024g+512; role Y owns rows
1024g+512..1024g+1024 and receives its embT with each adjacent 512-column
pair swapped so that, for both roles, key-chunk position 2g holds the core's
own q rows (diagonal chunk) and position 2g+1 holds the pad/keep chunk. The
SPMD program is identical across cores; all per-core differences live in the
input data (chunk order, triangle masks, exp bias).

Everything is bf16 on-chip (tolerance is 2e-2; bf16 keeps us ~1e-2 below
it): halves HBM traffic and doubles DVE throughput while the PE runs at the
same 1 row/cycle. Per q-group g the program processes n_k = 8g+8 key tiles
of 128. The first two kbs of the final 512-block pair are the diagonal:
multiplied by the shared triangle masks on DVE. The last two kbs are the
pad/keep chunk: handled for free by a per-partition bias in the exp
(role X: -30 => exp ~ 0, role Y: 0 => keep), so no mask data or multiply.

Layout trick (kept from the earlier version): everything is computed
transposed (d-major) so the softmax reduction is a free-dim reduction; the
host supplies emb^T. P^T = exp(S^T) feeds the PV matmul directly and the
softmax column-sum rides as a 65th row of a ones-augmented V.
"""

import sys

for _p in ("/opt/trn_rl_repo",):
    if _p not in sys.path:
        sys.path.insert(0, _p)

import numpy as np
import ml_dtypes

import concourse.bass as bass
import concourse.bacc as bacc
import concourse.mybir as mybir
from concourse.bass_utils import run_bass_kernel_spmd
from concourse.tile import TileContext

B, S, E, DK, DV = 4, 4096, 512, 64, 64
P = 128
NCORES = 8
NG = 4          # q-groups per core
QG = 512        # q rows per group
F32 = mybir.dt.float32
F32R = mybir.dt.float32r
BF16 = mybir.dt.bfloat16
EC = E // P     # 4 contraction chunks


def build_program():
    nc = bacc.Bacc("TRN2", target_bir_lowering=False, debug=False, num_devices=NCORES)

    embT = nc.declare_dram_parameter("embT", [E, S], BF16, isOutput=False)
    # wpack: cols c*192..c*192+128 = [Wk|Wv] chunk c, +128..+192 = Wq chunk c
    wpack = nc.declare_dram_parameter("wpack", [P, 6 * P], BF16, isOutput=False)
    # bvec: col 0 = [bk;bv], col 1 = [bq;0], col 2 = exp bias (0 or -30),
    # cols 3:131 = 128x128 identity (for PE transposes; hence f32r typing —
    # the BIR verifier requires fp32r matmul inputs to be fp32r-produced)
    bvec = nc.declare_dram_parameter("bvec", [P, 3 + P], F32R, isOutput=False)
    tri = nc.declare_dram_parameter("tri", [P, 4 * QG], BF16, isOutput=False)
    # flash-style output: unnormalized O^T (64 rows) + softmax colsum (row 65);
    # the host performs the final divide + transpose during unsharding
    out = nc.declare_dram_parameter("out", [NG, DV + 1, QG], F32, isOutput=True)

    with TileContext(nc) as tc:
        with (
            tc.tile_pool(name="singles", bufs=1) as singles,
            tc.tile_pool(name="pt", bufs=6) as pt_pool,
            tc.tile_pool(name="fin", bufs=2) as fin_pool,
            tc.tile_pool(name="ps_s", bufs=3, space="PSUM") as ps_s_pool,
            tc.tile_pool(name="ps_o", bufs=1, space="PSUM") as ps_o_pool,
            tc.tile_pool(name="ps_sm", bufs=1, space="PSUM") as ps_sm_pool,
        ):
            # ---- persistent tensors ----
            et = [singles.tile([P, EC, QG], BF16, name=f"et{j}") for j in range(2 * NG)]
            kvt = singles.tile([P, S], F32R)            # K^T rows 0:64, V^T rows 64:128
            qt = singles.tile([DK, NG * QG], F32R)      # Q^T
            vn = singles.tile([P, S // P, DK + 1], F32R)  # V natural + ones col
            tri_sb = singles.tile([P, 4 * QG], BF16)
            w_sb = singles.tile([P, 6 * P], BF16)
            b_sb = singles.tile([P, 3 + P], F32R)

            bkv_sb = b_sb[:, 0:1].bitcast(F32)
            bq_sb = b_sb[0:DK, 1:2].bitcast(F32)
            b0_sb = b_sb[:, 2:3].bitcast(F32)
            id_sb = b_sb[:, 3:3 + P]

            def wkv_c(c):
                return w_sb[:, c * 192:c * 192 + 2 * DK]

            def wq_c(c):
                return w_sb[:, c * 192 + 2 * DK:c * 192 + 3 * DK]

            # PE warmup: junk matmuls with no DMA dependency ramp the tensor
            # engine's p-state before the real prologue arrives
            wu = singles.tile([P, 2 * P], BF16)
            nc.vector.memset(wu, 1.0)
            wups = ps_sm_pool.tile([P, QG], F32, tag="sm")
            for _ in range(14):
                nc.tensor.matmul(wups[:, 0:2 * P], wu[:, 0:P], wu, start=True, stop=True)

            nc.vector.memset(vn[:, :, DK:DK + 1].bitcast(F32), 1.0)  # ones col for fused colsum

            def kv_proj(j, pool=None, tag="sm"):
                pkv = (pool or ps_sm_pool).tile([P, QG], F32, tag=tag, name=f"pkv{j}")
                for c in range(EC):
                    nc.tensor.matmul(
                        pkv, wkv_c(c), et[j][:, c, :],
                        start=(c == 0), stop=(c == EC - 1),
                    )
                nc.vector.tensor_scalar_add(kvt[:, j * QG:(j + 1) * QG], pkv, bkv_sb)

            def kv_vnat(j, pool=None, tag="sm"):
                # V natural: transpose V^T 128-token blocks
                pv = (pool or ps_sm_pool).tile([P, QG], F32R, tag=tag, name=f"pv{j}")
                for s in range(QG // P):
                    nc.tensor.transpose(
                        pv[:, s * DK:(s + 1) * DK],
                        kvt[DK:2 * DK, (j * 4 + s) * P:(j * 4 + s + 1) * P],
                        id_sb[DK:2 * DK, DK:2 * DK],
                    )
                nc.vector.tensor_copy(
                    vn[:, j * 4:(j + 1) * 4, 0:DK],
                    pv[:, 0:4 * DK].rearrange("p (s d) -> p s d", d=DK),
                )

            def q_proj(g):
                # q rows live in chunk 2g for both roles (role Y data is swapped)
                pq = ps_sm_pool.tile([P, QG], F32, tag="sm")
                for c in range(EC):
                    nc.tensor.matmul(
                        pq[0:DK, :], wq_c(c), et[2 * g][:, c, :],
                        start=(c == 0), stop=(c == EC - 1),
                    )
                nc.vector.tensor_scalar_add(qt[:, g * QG:(g + 1) * QG], pq[0:DK, :], bq_sb)

            def kt_of(g, kb, i):
                # natural order: kbs 0..n_kb-5 naturals, n_kb-4..n_kb-3 the
                # diagonal chunk (tri masks), n_kb-2..n_kb-1 pad (exp bias)
                return 2 * kb + i

            def diag_col0(g, kb, i):
                # diagonal tile kt' leaves q columns < kt'*128 fully masked;
                # compute only the live range (min width 256 — below that the
                # f32r PE drops to 1/4 rate and nothing is saved). Stale psum
                # in the skipped region is finite and never consumed.
                n_kb = 4 * g + 4
                if n_kb - 4 <= kb < n_kb - 2:
                    ktp = 2 * (kb - (n_kb - 4)) + i
                    return min(ktp * P, QG - 2 * P)
                return 0

            def scores(g, kb):
                ps = ps_s_pool.tile([P, 2 * QG], F32, tag="ps")
                qg = qt[:, g * QG:(g + 1) * QG]
                for i in range(2):
                    kt = kt_of(g, kb, i)
                    c0 = diag_col0(g, kb, i)
                    nc.tensor.matmul(
                        ps[:, i * QG + c0:(i + 1) * QG],
                        kvt[0:DK, kt * P:(kt + 1) * P], qg[:, c0:],
                        start=True, stop=True,
                    )
                return ps

            def attention(g, fillers=None, first_pss=None):
                fillers = dict(fillers or {})
                n_k = 8 * g + 8
                n_kb = n_k // 2
                po = ps_o_pool.tile([P, QG], F32, tag="po")

                # software pipeline: S^T is EMITTED two kbs ahead of its exp so
                # it sits ahead of PV(kb-1) in the in-order PE FIFO — with 3
                # score buffers the exp stream runs back-to-back
                pss = list(first_pss) if first_pss else [scores(g, 0), scores(g, 1)]
                next_pss = []
                for kb in range(n_kb):
                    pt = pt_pool.tile([P, 2 * QG], F32R, tag="pt")
                    # pad/keep chunk (last 2 kbs): zeroed or kept purely via
                    # the per-partition exp bias, no mask multiply needed
                    bias = b0_sb if kb >= n_kb - 2 else 0.0
                    if kb == n_kb - 3:
                        # second diagonal kb: both tiles only need q columns
                        # 256:512 (the rest is fully masked), one strided exp
                        nc.scalar.activation(
                            pt.rearrange("p (i q) -> p i q", q=QG)[:, :, 2 * P:],
                            pss[kb].rearrange("p (i q) -> p i q", q=QG)[:, :, 2 * P:],
                            mybir.ActivationFunctionType.Exp,
                            scale=0.125, bias=bias,
                        )
                    else:
                        nc.scalar.activation(
                            pt, pss[kb], mybir.ActivationFunctionType.Exp,
                            scale=0.125, bias=bias,
                        )
                    if len(pss) < n_kb:
                        pss.append(scores(g, len(pss)))
                    elif g + 1 < NG and len(next_pss) < 2:
                        # hoist the next group's first two S^T into the last
                        # two kbs so its exps start with no transition bubble
                        next_pss.append(scores(g + 1, len(next_pss)))
                    # upcoming prologue pieces ride in the PE FIFO ahead of
                    # this kb's PV, placed at the kb where their DMA data has
                    # already landed (an early slot would stall the FIFO)
                    for f in fillers.pop(kb, ()):
                        f()
                    for i in range(2):
                        kt = kt_of(g, kb, i)
                        c0 = diag_col0(g, kb, i)
                        if n_kb - 4 <= kb < n_kb - 2:
                            # diagonal chunk: per-tile triangle mul so PV(i)
                            # only waits on its own half. Group 0's masks run
                            # on DVE (gpsimd is still generating DMA descs);
                            # later groups use the idle gpsimd so the DVE
                            # FIFO never delays bias-adds feeding next scores
                            kk = 2 * (kb - (n_kb - 4)) + i
                            eng = nc.vector
                            eng.tensor_mul(
                                pt[:, i * QG + c0:(i + 1) * QG],
                                pt[:, i * QG + c0:(i + 1) * QG],
                                tri_sb[:, kk * QG + c0:(kk + 1) * QG],
                            )
                        nc.tensor.matmul(
                            po[0:DV + 1, c0:],
                            vn[:, kt, :], pt[:, i * QG + c0:(i + 1) * QG],
                            start=(kb == 0 and i == 0),
                            stop=(kb == n_kb - 1 and i == 1),
                        )
                for kb in sorted(fillers):
                    for f in fillers[kb]:
                        f()
                # finalize: copy O^T + colsum row out of PSUM and store raw;
                # the host divides by the colsum and transposes. The last
                # group splits in half so the copy overlaps the first store.
                nsp = 2 if g == NG - 1 else 1
                hw = QG // nsp
                for s in range(nsp):
                    if nsp == 2:
                        ot = fin_pool.tile([DV + 1, QG // 2], F32, tag="ot", name="ot_h")
                        # halves copied on different engines (Act is idle by
                        # now) so both stores issue ~simultaneously
                        if s == 0:
                            nc.vector.tensor_copy(ot, po[0:DV + 1, s * hw:(s + 1) * hw])
                        else:
                            nc.scalar.copy(ot, po[0:DV + 1, s * hw:(s + 1) * hw])
                    else:
                        ot = fin_pool.tile([DV + 1, QG], F32, tag="otf", name="ot_f")
                        nc.vector.tensor_copy(ot, po[0:DV + 1, s * hw:(s + 1) * hw])
                    nc.sync.dma_start(out=out[:][g][:, s * hw:(s + 1) * hw], in_=ot)
                return next_pss

            # emission schedule: small loads (weights/biases/masks) go via SP
            # whose HWDGE path frees the sequencer after desc-gen; the big
            # embT chunks stream via the otherwise-idle gpsimd SWDGE queue.
            def et_dma(j):
                nc.gpsimd.dma_start(
                    out=et[j],
                    in_=embT[:].rearrange("(c p) t -> p c t", p=P)[:, :, j * QG:(j + 1) * QG],
                )

            # first embT chunk lands in two token-half DMAs so the prologue's
            # projections start ~1.5 us earlier. Queue assignment: weights on
            # SP, the small bias vector on the idle Act queue, everything
            # else ordered on the gpsimd SWDGE queue (tri after et1 so the
            # big mask transfer can't jump ahead of the critical chunks).
            hw = QG // 2
            nc.sync.dma_start(out=w_sb, in_=wpack[:])
            nc.scalar.dma_start(out=b_sb, in_=bvec[:])
            for h in range(2):
                nc.gpsimd.dma_start(
                    out=et[0][:, :, h * hw:(h + 1) * hw],
                    in_=embT[:].rearrange("(c p) t -> p c t", p=P)[:, :, h * hw:(h + 1) * hw],
                )
            et_dma(1)
            et_dma(2)
            nc.gpsimd.dma_start(out=tri_sb[:, 0:2 * QG], in_=tri[:][:, 0:2 * QG])
            et_dma(3)
            nc.gpsimd.dma_start(out=tri_sb[:, 2 * QG:4 * QG], in_=tri[:][:, 2 * QG:4 * QG])
            for j in range(4, 2 * NG):
                et_dma(j)

            # prologue, token-halved to shorten the first-exp critical path:
            # Q and K/V chunk 0 alternate per half (Q psum borrows a score
            # buffer so the sm slot never serializes Q against K/V), then the
            # first two scores go ahead of the chunk-1 projection in the PE
            # FIFO. Chunk 1 still precedes attention kb0, whose scores(+2)
            # emission reads it; its halves let kb0's scores start after the
            # first half's bias-add.
            pq0 = ps_s_pool.tile([P, QG], F32, tag="ps", name="pq0")
            pkv0 = ps_sm_pool.tile([P, QG], F32, tag="sm")
            for h in range(2):
                for c in range(EC):
                    nc.tensor.matmul(
                        pq0[0:DK, h * hw:(h + 1) * hw], wq_c(c),
                        et[0][:, c, h * hw:(h + 1) * hw],
                        start=(c == 0), stop=(c == EC - 1),
                    )
                nc.vector.tensor_scalar_add(
                    qt[:, h * hw:(h + 1) * hw], pq0[0:DK, h * hw:(h + 1) * hw], bq_sb
                )
                for c in range(EC):
                    nc.tensor.matmul(
                        pkv0[:, h * hw:(h + 1) * hw], wkv_c(c),
                        et[0][:, c, h * hw:(h + 1) * hw],
                        start=(c == 0), stop=(c == EC - 1),
                    )
                # chunk-0 bias adds ride the idle Activation engine so the
                # DVE only carries the q-side adds before the first scores
                nc.scalar.add(
                    kvt[:, h * hw:(h + 1) * hw], pkv0[:, h * hw:(h + 1) * hw], bkv_sb
                )
            first_pss = [scores(0, 0), scores(0, 1)]
            pk1 = ps_s_pool.tile([P, QG], F32, tag="ps", name="pk1")
            for h in range(2):
                for c in range(EC):
                    nc.tensor.matmul(
                        pk1[:, h * hw:(h + 1) * hw], wkv_c(c),
                        et[1][:, c, h * hw:(h + 1) * hw],
                        start=(c == 0), stop=(c == EC - 1),
                    )
                nc.vector.tensor_scalar_add(
                    kvt[:, QG + h * hw:QG + (h + 1) * hw], pk1[:, h * hw:(h + 1) * hw], bkv_sb
                )
            kv_vnat(0)

            def KP(j):
                return lambda: kv_proj(j)

            def KV(j):
                return lambda: kv_vnat(j)

            def QP(g):
                return lambda: q_proj(g)

            slot_plan = {
                0: {0: [QP(1)], 1: [KV(1)], 2: [KP(2)], 3: [KP(3)]},
                1: {0: [KV(2)], 2: [KV(3)], 5: [QP(2)]},
                2: {0: [KP(4)], 1: [KV(4)], 3: [KP(5)], 5: [KV(5)], 7: [QP(3)]},
                3: {0: [KP(6)], 1: [KV(6)], 3: [KP(7)], 5: [KV(7)]},
            }
            nxt = first_pss
            for g in range(NG):
                nxt = attention(g, slot_plan[g], first_pss=nxt)

    nc.compile()
    return nc


_PROGRAM = None


def _get_program():
    global _PROGRAM
    if _PROGRAM is None:
        _PROGRAM = build_program()
    return _PROGRAM


def kernel(embedding_matrix, Wq_w, Wq_b, Wk_w, Wk_b, Wv_w, Wv_b):
    emb = np.asarray(embedding_matrix, dtype=np.float32)
    wq = np.asarray(Wq_w, np.float32)
    wk = np.asarray(Wk_w, np.float32)
    wv = np.asarray(Wv_w, np.float32)
    bq = np.asarray(Wq_b, np.float32)
    bk = np.asarray(Wk_b, np.float32)
    bv = np.asarray(Wv_b, np.float32)

    # wpack: per 128-row chunk c of E: [Wk|Wv|Wq]
    wqkv = np.concatenate([wk, wv, wq], axis=1).reshape(EC, P, 3 * DK)  # [4,128,192]
    wpack = np.empty((P, 6 * P), np.float32)
    for c in range(EC):
        wpack[:, c * 192:(c + 1) * 192] = wqkv[c]
    wpack = wpack.astype(ml_dtypes.bfloat16)

    # shared diagonal triangle: keep iff kt*128 + p <= j (within 512 block)
    pp = np.arange(P)[:, None]
    jj = np.arange(QG)[None, :]
    trim = np.zeros((P, 4 * QG), np.float32)
    for kt in range(4):
        trim[:, kt * QG:(kt + 1) * QG] = ((pp + kt * P) <= jj).astype(np.float32)
    trim = trim.astype(ml_dtypes.bfloat16)

    bvec_by_role = []
    for role in range(2):
        bv3 = np.zeros((P, 3 + P), np.float32)
        bv3[:, 0] = np.concatenate([bk, bv])
        bv3[0:DK, 1] = bq
        bv3[:, 2] = -30.0 if role == 0 else 0.0
        bv3[:, 3:3 + P] = np.eye(P, dtype=np.float32)
        bvec_by_role.append(bv3)

    emb_bf = emb.astype(ml_dtypes.bfloat16)
    in_maps = []
    for c in range(NCORES):
        b, role = c // 2, c % 2
        if role == 0:
            # role X: natural order, q rows = 1024g..1024g+512 (chunk 2g)
            ebT = np.ascontiguousarray(emb_bf[b].T)
        else:
            # role Y: swap adjacent 512-blocks so q rows land at chunk 2g
            sw = emb_bf[b].reshape(NG, 2, QG, E)[:, ::-1].reshape(S, E)
            ebT = np.ascontiguousarray(sw.T)
        in_maps.append({
            "embT": ebT, "wpack": wpack, "bvec": bvec_by_role[role], "tri": trim,
        })

    nc = _get_program()
    results = run_bass_kernel_spmd(nc, in_maps, list(range(NCORES))).results

    out = np.empty((B, S, DV), np.float32)
    for c in range(NCORES):
        b, role = c // 2, c % 2
        o = results[c]["out"]                                   # [NG, 65, 512]
        for g in range(NG):
            q0 = 1024 * g + (0 if role == 0 else QG)
            out[b, q0:q0 + QG] = (o[g, 0:DV, :] / o[g, DV:DV + 1, :]).T
    return out


if __name__ == "__main__":
    rng = np.random.default_rng(0)
    ins = {
        "embedding_matrix": rng.standard_normal((B, S, E), dtype=np.float32),
        "Wq_w": rng.standard_normal((E, DK), dtype=np.float32) * 0.04,
        "Wq_b": rng.standard_normal((DK,), dtype=np.float32) * 0.04,
        "Wk_w": rng.standard_normal((E, DK), dtype=np.float32) * 0.04,
        "Wk_b": rng.standard_normal((DK,), dtype=np.float32) * 0.04,
        "Wv_w": rng.standard_normal((E, DV), dtype=np.float32) * 0.04,
        "Wv_b": rng.standard_normal((DV,), dtype=np.float32) * 0.04,
    }
    o = kernel(**ins)
    print("kernel ran, out:", o.shape, o.dtype, float(np.abs(o).max()))


# revision 79
# speedup vs baseline: 1.0282x; 1.0282x over previous
"""Single-head causal attention (B=4, S=4096, E=512, DK=DV=64) on 8 trn2 cores.

Sharding: 2 cores per batch element; each core owns 4 q-groups of 512 rows.
Core role X (natural key order) owns rows 1024g..1024g+512; role Y owns rows
1024g+512..1024g+1024 and receives its embT with each adjacent 512-column
pair swapped so that, for both roles, key-chunk position 2g holds the core's
own q rows (diagonal chunk) and position 2g+1 holds the pad/keep chunk. The
SPMD program is identical across cores; all per-core differences live in the
input data (chunk order, triangle masks, exp bias).

Everything is bf16 on-chip (tolerance is 2e-2; bf16 keeps us ~1e-2 below
it): halves HBM traffic and doubles DVE throughput while the PE runs at the
same 1 row/cycle. Per q-group g the program processes n_k = 8g+8 key tiles
of 128. The first two kbs of the final 512-block pair are the diagonal:
multiplied by the shared triangle masks on DVE. The last two kbs are the
pad/keep chunk: handled for free by a per-partition bias in the exp
(role X: -30 => exp ~ 0, role Y: 0 => keep), so no mask data or multiply.

Layout trick (kept from the earlier version): everything is computed
transposed (d-major) so the softmax reduction is a free-dim reduction; the
host supplies emb^T. P^T = exp(S^T) feeds the PV matmul directly and the
softmax column-sum rides as a 65th row of a ones-augmented V.
"""

import sys

for _p in ("/opt/trn_rl_repo",):
    if _p not in sys.path:
        sys.path.insert(0, _p)

import numpy as np
import ml_dtypes

import concourse.bass as bass
import concourse.bacc as bacc
import concourse.mybir as mybir
from concourse.bass_utils import run_bass_kernel_spmd
from concourse.tile import TileContext

B, S, E, DK, DV = 4, 4096, 512, 64, 64
P = 128
NCORES = 8
NG = 4          # q-groups per core
QG = 512        # q rows per group
F32 = mybir.dt.float32
F32R = mybir.dt.float32r
BF16 = mybir.dt.bfloat16
EC = E // P     # 4 contraction chunks


def build_program():
    nc = bacc.Bacc("TRN2", target_bir_lowering=False, debug=False, num_devices=NCORES)

    embT = nc.declare_dram_parameter("embT", [E, S], BF16, isOutput=False)
    # wpack: cols c*192..c*192+128 = [Wk|Wv] chunk c, +128..+192 = Wq chunk c
    wpack = nc.declare_dram_parameter("wpack", [P, 6 * P], BF16, isOutput=False)
    # bvec: col 0 = [bk;bv], col 1 = [bq;0], col 2 = exp bias (0 or -30),
    # cols 3:131 = 128x128 identity (for PE transposes; hence f32r typing —
    # the BIR verifier requires fp32r matmul inputs to be fp32r-produced)
    bvec = nc.declare_dram_parameter("bvec", [P, 3 + P], F32R, isOutput=False)
    tri = nc.declare_dram_parameter("tri", [P, 4 * QG], BF16, isOutput=False)
    # flash-style output: unnormalized O^T (64 rows) + softmax colsum (row 65);
    # the host performs the final divide + transpose during unsharding
    out = nc.declare_dram_parameter("out", [NG, DV + 1, QG], F32, isOutput=True)

    with TileContext(nc) as tc:
        with (
            tc.tile_pool(name="singles", bufs=1) as singles,
            tc.tile_pool(name="pt", bufs=6) as pt_pool,
            tc.tile_pool(name="fin", bufs=2) as fin_pool,
            tc.tile_pool(name="ps_s", bufs=3, space="PSUM") as ps_s_pool,
            tc.tile_pool(name="ps_o", bufs=1, space="PSUM") as ps_o_pool,
            tc.tile_pool(name="ps_sm", bufs=1, space="PSUM") as ps_sm_pool,
        ):
            # ---- persistent tensors ----
            et = [singles.tile([P, EC, QG], BF16, name=f"et{j}") for j in range(2 * NG)]
            kvt = singles.tile([P, S], F32R)            # K^T rows 0:64, V^T rows 64:128
            qt = singles.tile([DK, NG * QG], F32R)      # Q^T
            vn = singles.tile([P, S // P, DK + 1], F32R)  # V natural + ones col
            tri_sb = singles.tile([P, 4 * QG], BF16)
            w_sb = singles.tile([P, 6 * P], BF16)
            b_sb = singles.tile([P, 3 + P], F32R)

            bkv_sb = b_sb[:, 0:1].bitcast(F32)
            bq_sb = b_sb[0:DK, 1:2].bitcast(F32)
            b0_sb = b_sb[:, 2:3].bitcast(F32)
            id_sb = b_sb[:, 3:3 + P]

            def wkv_c(c):
                return w_sb[:, c * 192:c * 192 + 2 * DK]

            def wq_c(c):
                return w_sb[:, c * 192 + 2 * DK:c * 192 + 3 * DK]

            # PE warmup: junk matmuls with no DMA dependency ramp the tensor
            # engine's p-state before the real prologue arrives
            wu = singles.tile([P, 2 * P], BF16)
            nc.vector.memset(wu, 1.0)
            wups = ps_sm_pool.tile([P, QG], F32, tag="sm")
            for _ in range(14):
                nc.tensor.matmul(wups[:, 0:2 * P], wu[:, 0:P], wu, start=True, stop=True)

            nc.vector.memset(vn[:, :, DK:DK + 1].bitcast(F32), 1.0)  # ones col for fused colsum

            def kv_proj(j, pool=None, tag="sm"):
                pkv = (pool or ps_sm_pool).tile([P, QG], F32, tag=tag, name=f"pkv{j}")
                for c in range(EC):
                    nc.tensor.matmul(
                        pkv, wkv_c(c), et[j][:, c, :],
                        start=(c == 0), stop=(c == EC - 1),
                    )
                nc.vector.tensor_scalar_add(kvt[:, j * QG:(j + 1) * QG], pkv, bkv_sb)

            def kv_vnat(j, pool=None, tag="sm"):
                # V natural: transpose V^T 128-token blocks
                pv = (pool or ps_sm_pool).tile([P, QG], F32R, tag=tag, name=f"pv{j}")
                for s in range(QG // P):
                    nc.tensor.transpose(
                        pv[:, s * DK:(s + 1) * DK],
                        kvt[DK:2 * DK, (j * 4 + s) * P:(j * 4 + s + 1) * P],
                        id_sb[DK:2 * DK, DK:2 * DK],
                    )
                nc.vector.tensor_copy(
                    vn[:, j * 4:(j + 1) * 4, 0:DK],
                    pv[:, 0:4 * DK].rearrange("p (s d) -> p s d", d=DK),
                )

            def q_proj(g):
                # q rows live in chunk 2g for both roles (role Y data is swapped)
                pq = ps_sm_pool.tile([P, QG], F32, tag="sm")
                for c in range(EC):
                    nc.tensor.matmul(
                        pq[0:DK, :], wq_c(c), et[2 * g][:, c, :],
                        start=(c == 0), stop=(c == EC - 1),
                    )
                nc.vector.tensor_scalar_add(qt[:, g * QG:(g + 1) * QG], pq[0:DK, :], bq_sb)

            def kt_of(g, kb, i):
                # natural order: kbs 0..n_kb-5 naturals, n_kb-4..n_kb-3 the
                # diagonal chunk (tri masks), n_kb-2..n_kb-1 pad (exp bias)
                return 2 * kb + i

            def diag_col0(g, kb, i):
                # diagonal tile kt' leaves q columns < kt'*128 fully masked;
                # compute only the live range (min width 256 — below that the
                # f32r PE drops to 1/4 rate and nothing is saved). Stale psum
                # in the skipped region is finite and never consumed.
                n_kb = 4 * g + 4
                if n_kb - 4 <= kb < n_kb - 2:
                    ktp = 2 * (kb - (n_kb - 4)) + i
                    return min(ktp * P, QG - 2 * P)
                return 0

            def scores(g, kb):
                ps = ps_s_pool.tile([P, 2 * QG], F32, tag="ps")
                qg = qt[:, g * QG:(g + 1) * QG]
                for i in range(2):
                    kt = kt_of(g, kb, i)
                    c0 = diag_col0(g, kb, i)
                    nc.tensor.matmul(
                        ps[:, i * QG + c0:(i + 1) * QG],
                        kvt[0:DK, kt * P:(kt + 1) * P], qg[:, c0:],
                        start=True, stop=True,
                    )
                return ps

            def attention(g, fillers=None, first_pss=None):
                fillers = dict(fillers or {})
                n_k = 8 * g + 8
                n_kb = n_k // 2
                po = ps_o_pool.tile([P, QG], F32, tag="po")

                # software pipeline: S^T is EMITTED two kbs ahead of its exp so
                # it sits ahead of PV(kb-1) in the in-order PE FIFO — with 3
                # score buffers the exp stream runs back-to-back
                pss = list(first_pss) if first_pss else [scores(g, 0), scores(g, 1)]
                next_pss = []
                for kb in range(n_kb):
                    pt = pt_pool.tile([P, 2 * QG], F32R, tag="pt")
                    # pad/keep chunk (last 2 kbs): zeroed or kept purely via
                    # the per-partition exp bias, no mask multiply needed
                    bias = b0_sb if kb >= n_kb - 2 else 0.0
                    if kb == n_kb - 3:
                        # second diagonal kb: both tiles only need q columns
                        # 256:512 (the rest is fully masked), one strided exp
                        nc.scalar.activation(
                            pt.rearrange("p (i q) -> p i q", q=QG)[:, :, 2 * P:],
                            pss[kb].rearrange("p (i q) -> p i q", q=QG)[:, :, 2 * P:],
                            mybir.ActivationFunctionType.Exp,
                            scale=0.125, bias=bias,
                        )
                    else:
                        nc.scalar.activation(
                            pt, pss[kb], mybir.ActivationFunctionType.Exp,
                            scale=0.125, bias=bias,
                        )
                    if len(pss) < n_kb:
                        pss.append(scores(g, len(pss)))
                    elif g + 1 < NG and len(next_pss) < 2:
                        # hoist the next group's first two S^T into the last
                        # two kbs so its exps start with no transition bubble
                        next_pss.append(scores(g + 1, len(next_pss)))
                    # upcoming prologue pieces ride in the PE FIFO ahead of
                    # this kb's PV, placed at the kb where their DMA data has
                    # already landed (an early slot would stall the FIFO)
                    for f in fillers.pop(kb, ()):
                        f()
                    for i in range(2):
                        kt = kt_of(g, kb, i)
                        c0 = diag_col0(g, kb, i)
                        if n_kb - 4 <= kb < n_kb - 2:
                            # diagonal chunk: per-tile triangle mul so PV(i)
                            # only waits on its own half. Group 0's masks run
                            # on DVE (gpsimd is still generating DMA descs);
                            # later groups use the idle gpsimd so the DVE
                            # FIFO never delays bias-adds feeding next scores
                            kk = 2 * (kb - (n_kb - 4)) + i
                            eng = nc.vector
                            eng.tensor_mul(
                                pt[:, i * QG + c0:(i + 1) * QG],
                                pt[:, i * QG + c0:(i + 1) * QG],
                                tri_sb[:, kk * QG + c0:(kk + 1) * QG],
                            )
                        nc.tensor.matmul(
                            po[0:DV + 1, c0:],
                            vn[:, kt, :], pt[:, i * QG + c0:(i + 1) * QG],
                            start=(kb == 0 and i == 0),
                            stop=(kb == n_kb - 1 and i == 1),
                        )
                for kb in sorted(fillers):
                    for f in fillers[kb]:
                        f()
                # finalize: copy O^T + colsum row out of PSUM and store raw;
                # the host divides by the colsum and transposes. The last
                # group splits in half so the copy overlaps the first store.
                nsp = 2 if g == NG - 1 else 1
                hw = QG // nsp
                for s in range(nsp):
                    if nsp == 2:
                        ot = fin_pool.tile([DV + 1, QG // 2], F32, tag="ot", name="ot_h")
                        # halves copied on different engines (Act is idle by
                        # now) so both stores issue ~simultaneously
                        if s == 0:
                            nc.vector.tensor_copy(ot, po[0:DV + 1, s * hw:(s + 1) * hw])
                        else:
                            nc.scalar.copy(ot, po[0:DV + 1, s * hw:(s + 1) * hw])
                    else:
                        ot = fin_pool.tile([DV + 1, QG], F32, tag="otf", name="ot_f")
                        nc.vector.tensor_copy(ot, po[0:DV + 1, s * hw:(s + 1) * hw])
                    nc.sync.dma_start(out=out[:][g][:, s * hw:(s + 1) * hw], in_=ot)
                return next_pss

            # emission schedule: small loads (weights/biases/masks) go via SP
            # whose HWDGE path frees the sequencer after desc-gen; the big
            # embT chunks stream via the otherwise-idle gpsimd SWDGE queue.
            def et_dma(j):
                nc.gpsimd.dma_start(
                    out=et[j],
                    in_=embT[:].rearrange("(c p) t -> p c t", p=P)[:, :, j * QG:(j + 1) * QG],
                )

            # first embT chunk lands in two token-half DMAs so the prologue's
            # projections start ~1.5 us earlier. Queue assignment: weights on
            # SP, the small bias vector on the idle Act queue, everything
            # else ordered on the gpsimd SWDGE queue (tri after et1 so the
            # big mask transfer can't jump ahead of the critical chunks).
            hw = QG // 2
            nc.sync.dma_start(out=w_sb, in_=wpack[:])
            nc.scalar.dma_start(out=b_sb, in_=bvec[:])
            for h in range(2):
                nc.gpsimd.dma_start(
                    out=et[0][:, :, h * hw:(h + 1) * hw],
                    in_=embT[:].rearrange("(c p) t -> p c t", p=P)[:, :, h * hw:(h + 1) * hw],
                )
            et_dma(1)
            et_dma(2)
            nc.gpsimd.dma_start(out=tri_sb[:, 0:2 * QG], in_=tri[:][:, 0:2 * QG])
            et_dma(3)
            nc.gpsimd.dma_start(out=tri_sb[:, 2 * QG:4 * QG], in_=tri[:][:, 2 * QG:4 * QG])
            for j in range(4, 2 * NG):
                et_dma(j)

            # prologue, token-halved to shorten the first-exp critical path:
            # Q and K/V chunk 0 alternate per half (Q psum borrows a score
            # buffer so the sm slot never serializes Q against K/V), then the
            # first two scores go ahead of the chunk-1 projection in the PE
            # FIFO. Chunk 1 still precedes attention kb0, whose scores(+2)
            # emission reads it; its halves let kb0's scores start after the
            # first half's bias-add.
            pq0 = ps_s_pool.tile([P, QG], F32, tag="ps", name="pq0")
            pkv0 = ps_sm_pool.tile([P, QG], F32, tag="sm")
            for h in range(2):
                for c in range(EC):
                    nc.tensor.matmul(
                        pq0[0:DK, h * hw:(h + 1) * hw], wq_c(c),
                        et[0][:, c, h * hw:(h + 1) * hw],
                        start=(c == 0), stop=(c == EC - 1),
                    )
                nc.vector.tensor_scalar_add(
                    qt[:, h * hw:(h + 1) * hw], pq0[0:DK, h * hw:(h + 1) * hw], bq_sb
                )
                for c in range(EC):
                    nc.tensor.matmul(
                        pkv0[:, h * hw:(h + 1) * hw], wkv_c(c),
                        et[0][:, c, h * hw:(h + 1) * hw],
                        start=(c == 0), stop=(c == EC - 1),
                    )
                nc.vector.tensor_scalar_add(
                    kvt[:, h * hw:(h + 1) * hw], pkv0[:, h * hw:(h + 1) * hw], bkv_sb
                )
            first_pss = [scores(0, 0), scores(0, 1)]
            pk1 = ps_s_pool.tile([P, QG], F32, tag="ps", name="pk1")
            for h in range(2):
                for c in range(EC):
                    nc.tensor.matmul(
                        pk1[:, h * hw:(h + 1) * hw], wkv_c(c),
                        et[1][:, c, h * hw:(h + 1) * hw],
                        start=(c == 0), stop=(c == EC - 1),
                    )
                nc.vector.tensor_scalar_add(
                    kvt[:, QG + h * hw:QG + (h + 1) * hw], pk1[:, h * hw:(h + 1) * hw], bkv_sb
                )
            kv_vnat(0)

            def KP(j):
                return lambda: kv_proj(j)

            def KV(j):
                return lambda: kv_vnat(j)

            def QP(g):
                return lambda: q_proj(g)

            slot_plan = {
                0: {0: [QP(1)], 1: [KV(1)], 2: [KP(2)], 3: [KP(3)]},
                1: {0: [KV(2)], 2: [KV(3)], 5: [QP(2)]},
                2: {0: [KP(4)], 1: [KV(4)], 3: [KP(5)], 5: [KV(5)], 7: [QP(3)]},
                3: {0: [KP(6)], 1: [KV(6)], 3: [KP(7)], 5: [KV(7)]},
            }
            nxt = first_pss
            for g in range(NG):
                nxt = attention(g, slot_plan[g], first_pss=nxt)

    nc.compile()
    return nc


_PROGRAM = None


def _get_program():
    global _PROGRAM
    if _PROGRAM is None:
        _PROGRAM = build_program()
    return _PROGRAM


def kernel(embedding_matrix, Wq_w, Wq_b, Wk_w, Wk_b, Wv_w, Wv_b):
    emb = np.asarray(embedding_matrix, dtype=np.float32)
    wq = np.asarray(Wq_w, np.float32)
    wk = np.asarray(Wk_w, np.float32)
    wv = np.asarray(Wv_w, np.float32)
    bq = np.asarray(Wq_b, np.float32)
    bk = np.asarray(Wk_b, np.float32)
    bv = np.asarray(Wv_b, np.float32)

    # wpack: per 128-row chunk c of E: [Wk|Wv|Wq]
    wqkv = np.concatenate([wk, wv, wq], axis=1).reshape(EC, P, 3 * DK)  # [4,128,192]
    wpack = np.empty((P, 6 * P), np.float32)
    for c in range(EC):
        wpack[:, c * 192:(c + 1) * 192] = wqkv[c]
    wpack = wpack.astype(ml_dtypes.bfloat16)

    # shared diagonal triangle: keep iff kt*128 + p <= j (within 512 block)
    pp = np.arange(P)[:, None]
    jj = np.arange(QG)[None, :]
    trim = np.zeros((P, 4 * QG), np.float32)
    for kt in range(4):
        trim[:, kt * QG:(kt + 1) * QG] = ((pp + kt * P) <= jj).astype(np.float32)
    trim = trim.astype(ml_dtypes.bfloat16)

    bvec_by_role = []
    for role in range(2):
        bv3 = np.zeros((P, 3 + P), np.float32)
        bv3[:, 0] = np.concatenate([bk, bv])
        bv3[0:DK, 1] = bq
        bv3[:, 2] = -30.0 if role == 0 else 0.0
        bv3[:, 3:3 + P] = np.eye(P, dtype=np.float32)
        bvec_by_role.append(bv3)

    emb_bf = emb.astype(ml_dtypes.bfloat16)
    in_maps = []
    for c in range(NCORES):
        b, role = c // 2, c % 2
        if role == 0:
            # role X: natural order, q rows = 1024g..1024g+512 (chunk 2g)
            ebT = np.ascontiguousarray(emb_bf[b].T)
        else:
            # role Y: swap adjacent 512-blocks so q rows land at chunk 2g
            sw = emb_bf[b].reshape(NG, 2, QG, E)[:, ::-1].reshape(S, E)
            ebT = np.ascontiguousarray(sw.T)
        in_maps.append({
            "embT": ebT, "wpack": wpack, "bvec": bvec_by_role[role], "tri": trim,
        })

    nc = _get_program()
    results = run_bass_kernel_spmd(nc, in_maps, list(range(NCORES))).results

    out = np.empty((B, S, DV), np.float32)
    for c in range(NCORES):
        b, role = c // 2, c % 2
        o = results[c]["out"]                                   # [NG, 65, 512]
        for g in range(NG):
            q0 = 1024 * g + (0 if role == 0 else QG)
            out[b, q0:q0 + QG] = (o[g, 0:DV, :] / o[g, DV:DV + 1, :]).T
    return out


if __name__ == "__main__":
    rng = np.random.default_rng(0)
    ins = {
        "embedding_matrix": rng.standard_normal((B, S, E), dtype=np.float32),
        "Wq_w": rng.standard_normal((E, DK), dtype=np.float32) * 0.04,
        "Wq_b": rng.standard_normal((DK,), dtype=np.float32) * 0.04,
        "Wk_w": rng.standard_normal((E, DK), dtype=np.float32) * 0.04,
        "Wk_b": rng.standard_normal((DK,), dtype=np.float32) * 0.04,
        "Wv_w": rng.standard_normal((E, DV), dtype=np.float32) * 0.04,
        "Wv_b": rng.standard_normal((DV,), dtype=np.float32) * 0.04,
    }
    o = kernel(**ins)
    print("kernel ran, out:", o.shape, o.dtype, float(np.abs(o).max()))
